# revision 18
# baseline (speedup 1.0000x reference)
"""NeuralGCDE on 8 NeuronCores: full RK4 ODE integration on device.

Sharding: data-parallel over batch B=16 -> 2 batch elements per core
(rows r = b*N + n, R = 1024 per core). All graph/MLP params replicated.

Device layout is feature-major (features on SBUF partitions, rows on the
free dim). Every contraction is a PE matmul; partition-dim reductions and
broadcasts use structured 0/1 matrices as stationary operands. The
softmax adjacency (exp(relu(gE gE^T)) with row scaling) is built on
device; only the row-sum reciprocals (512 floats) come from host.

All one-time work (bass build, neuron compile, PJRT load) happens at
import; kernel(**inputs) does host repacks + one SPMD dispatch.
"""
import ml_dtypes
import numpy as np

import concourse.bass as bass
import concourse.mybir as mybir
import concourse.tile as tile

B, N, T = 16, 512, 12
IN, HID, HH, EMB, K, OUT = 2, 32, 32, 16, 2, 12
NCORES = 8
BS = B // NCORES            # 2
R = BS * N                  # 1024
NSTEP = T - 1               # 11
F32 = mybir.dt.float32
AF = mybir.ActivationFunctionType
ALU = mybir.AluOpType

# (name, shape) of every shared parameter, packed flat into one upload
_WSPEC = [
    ("recip", (128, 4)), ("gET", (EMB, N)), ("gbpool", (EMB, HH)),
    ("Wh", (IN, HID)), ("bh", (HID, 1)), ("Wz", (IN, HID)), ("bz", (HID, 1)),
    ("fw1", (HID, HH)), ("fb1", (HH, 1)), ("fw2", (HH, HH)), ("fb2", (HH, 1)),
    ("fw3", (HH, HID * IN)), ("fb3", (HID * IN, 1)),
    ("gw1", (HID, HH)), ("gb1", (HH, 1)),
    ("wpk0", (HH, EMB * HH)), ("wpk1", (HH, EMB * HH)),
    ("gwo", (HH, HID * HID)), ("gbo", (128, 8)),
    ("cw", (HID, OUT)), ("cb", (OUT, 1)),
]
_WSIZE = sum(int(np.prod(s)) for _, s in _WSPEC)
_WPAD = ((_WSIZE + 7) // 8) * 8
_WSH = _WPAD // 8

_NO_SPILL = {"InstEventSemaphore", "InstUnconditionalBranch",
             "InstConditionalBranch"}


def _spill_excess_waits(nc):
    """Walrus ISA structs hold one sync-wait slot on most instructions.
    Tile can emit several. Move excess waits onto InstEventSemaphore
    carriers inserted just before, on the same engine (waiting earlier on
    the same engine stream is always sound)."""
    nspill = 0
    for f in nc.m.functions:
        for blk in f.blocks:
            lst = blk.instructions
            i = 0
            while i < len(lst):
                ins = lst[i]
                si = ins.sync_info
                if (type(ins).__name__ in _NO_SPILL or si is None
                        or not si.on_wait or len(si.on_wait) <= 1):
                    i += 1
                    continue
                waits = list(si.on_wait)
                keep, excess = waits[-1:], waits[:-1]
                ins.sync_info = mybir.SyncInfo(on_wait=keep,
                                               on_update=list(si.on_update))
                carriers = []
                while excess:
                    chunk, excess = excess[:2], excess[2:]
                    es = mybir.InstEventSemaphore(
                        name=f"Wspill-{nspill}", ins=[], outs=[])
                    nspill += 1
                    es.engine = ins.engine
                    es.sync_info = mybir.SyncInfo(on_wait=chunk, on_update=[])
                    carriers.append(es)
                for k_, es in enumerate(carriers):
                    lst.insert(i + k_, es)
                i += len(carriers) + 1
    return nspill


def build_nc(nstep=NSTEP):
    nc = bass.Bass()

    def dp(name, shape, out=False):
        return nc.declare_dram_parameter(name, list(shape), F32, isOutput=out)

    NU = 3 * nstep + 1                     # unique dX stage rows
    BF16 = mybir.dt.bfloat16
    d_pc = nc.declare_dram_parameter("pc", [NU * IN, R], BF16, isOutput=False)
    d_x0 = dp("x0", (IN, R))
    d_wb = nc.declare_dram_parameter("wb", [_WSH], F32, isOutput=False)
    wb_in = nc.dram_tensor("wb_in", [_WSH], F32)
    wb_all = nc.dram_tensor("wb_all", [_WPAD], F32)
    d_out = nc.declare_dram_parameter("out", [OUT, R], mybir.dt.bfloat16,
                                      isOutput=True)

    C5 = 512  # fp32 moving-operand free-dim limit

    from contextlib import ExitStack
    with ExitStack() as es:
        tc = es.enter_context(tile.TileContext(nc))
        sgl = es.enter_context(tc.tile_pool(name="sgl", bufs=1))
        wrk = es.enter_context(tc.tile_pool(name="wrk", bufs=1))
        big1 = es.enter_context(tc.tile_pool(name="big1", bufs=1))
        big2 = es.enter_context(tc.tile_pool(name="big2", bufs=2))
        dxp = es.enter_context(tc.tile_pool(name="dxp", bufs=2))
        pA = es.enter_context(tc.tile_pool(name="pA", bufs=2, space="PSUM"))
        pB = es.enter_context(tc.tile_pool(name="pB", bufs=1, space="PSUM"))
        pT = es.enter_context(tc.tile_pool(name="pT", bufs=2, space="PSUM"))

        # gather the weight blob: each core uploaded 1/8th
        nc.sync.dma_start(out=wb_in[:], in_=d_wb[:])
        nc.gpsimd.collective_compute(
            "AllGather", ALU.bypass,
            replica_groups=[list(range(NCORES))],
            ins=[wb_in[:]], outs=[wb_all[:]])

        woff = [0]

        def load(name, shape):
            p_, f_ = shape
            t = sgl.tile([p_, f_], F32, tag=name, name=name)
            nc.sync.dma_start(
                out=t[:],
                in_=wb_all[woff[0]:woff[0] + p_ * f_].rearrange(
                    "(p f) -> p f", p=p_))
            woff[0] += p_ * f_
            return t

        W = {nm: load(nm, sh) for nm, sh in _WSPEC}
        (recip, gET, gbpool, Wh, bh, Wz, bz, fw1, fb1, fw2, fb2, fw3, fb3,
         gw1, gb1, wpk0, wpk1, gwo, gbo, cw, cb) = (
            W[nm] for nm, _ in _WSPEC)
        x0 = sgl.tile([IN, R], F32, tag="x0", name="x0")
        nc.sync.dma_start(out=x0[:], in_=d_x0[:])

        # ---- structured 0/1 matrices, built in place ----
        NE = ALU.not_equal

        def zeros_tile(name, shape):
            t = sgl.tile(list(shape), F32, tag=name, name=name)
            nc.gpsimd.memset(t[:], 0.0)
            return t

        def aff(t, ap, pattern, base=0, cm=0):
            nc.gpsimd.affine_select(out=ap, in_=ap, compare_op=NE, fill=1.0,
                                    base=base, pattern=pattern,
                                    channel_multiplier=cm)

        ident = zeros_tile("ident", (128, 128))
        aff(ident, ident[:], [[-1, 128]], cm=1)
        I32 = zeros_tile("I32", (HH, HH))
        aff(I32, I32[:], [[-1, HH]], cm=1)
        Bc = sgl.tile([IN, IN * HID], BF16, tag="Bc", name="Bc")
        nc.gpsimd.memset(Bc[:], 0.0)
        aff(Bc, Bc[:].rearrange("p (j y) -> p j y", y=HID), [[-1, IN], [0, HID]],
            cm=1)
        Erep = zeros_tile("Erep", (HID, 128))      # 1 iff col%32 == p
        aff(Erep, Erep[:].rearrange("p (j y) -> p j y", y=HID),
            [[0, 4], [-1, HID]], cm=1)
        S3 = zeros_tile("S3", (IN * HID, HID))     # 1 iff p%32 == col
        aff(S3, S3[:], [[-1, HID]], cm=1)
        aff(S3, S3[:], [[-1, HID]], base=-HID, cm=1)
        S2 = zeros_tile("S2", (128, HH))           # 1 iff p%32 == col
        for q in range(4):
            aff(S2, S2[:], [[-1, HH]], base=-q * HH, cm=1)
        Gsel = zeros_tile("Gsel", (EMB, 4 * 128))  # 1 iff col//32 == p
        aff(Gsel, Gsel[:].rearrange("p (j y) -> p j y", y=32),
            [[-1, EMB], [0, 32]], cm=1)
        # Sdz[p, j*32+y] = 1 iff y == 4j + p//32, composed as E4.T @ Cdz
        E4 = zeros_tile("E4", (4, 128))            # 1 iff col//32 == p
        aff(E4, E4[:].rearrange("p (j y) -> p j y", y=32), [[-1, 4], [0, 32]],
            cm=1)
        Cdz = zeros_tile("Cdz", (4, 8 * HID))      # 1 iff y == 4j + p
        aff(Cdz, Cdz[:].rearrange("p (j y) -> p j y", y=HID),
            [[4, 8], [-1, HID]], cm=1)
        sdzp = pA.tile([128, 8 * HID], F32, tag="mm", name="mm")
        nc.tensor.matmul(sdzp[:], E4[:], Cdz[:], start=True, stop=True)
        Sdz = sgl.tile([128, 8 * HID], F32, tag="Sdz", name="Sdz")
        nc.scalar.copy(Sdz[:], sdzp[:])

        # ---- abT[o, n] = (gE @ gbpool).T, used for both batch halves ----
        abp = pA.tile([HH, N], F32, tag="mm", name="mm")
        nc.tensor.matmul(abp[:], gbpool[:], gET[:], start=True, stop=True)
        abT = sgl.tile([HH, N], F32, tag="abT", name="abT")
        nc.scalar.copy(abT[:], abp[:])

        def mm2(ps, lhsT, rhs, start=True, stop=True):
            for c in range(2):
                nc.tensor.matmul(ps[:, c * C5:(c + 1) * C5], lhsT,
                                 rhs[:, c * C5:(c + 1) * C5],
                                 start=start, stop=stop)

        def act(out, in_, func, bias=0.0):
            nc.scalar.activation(out, in_, func, bias=bias)

        # ---- adjacency: expG chunks (exp(relu(gE gE^T)), m-major) ----
        expG = []
        for i in range(4):
            gp = pA.tile([128, N], F32, tag="mm", name="mm")
            nc.tensor.matmul(gp[:], gET[:, i * 128:(i + 1) * 128], gET[:],
                             start=True, stop=True)
            eg = sgl.tile([128, N], F32, tag=f"expG{i}", name=f"expG{i}")
            act(eg[:], gp[:], AF.Relu)
            act(eg[:], eg[:], AF.Exp)
            expG.append(eg)

        # ---- gE_part chunks: gEp_j[p, r] = gE[n(r), (j*128+p)//32] ----
        gEp = []
        for j in range(4):
            ps = pA.tile([128, R], F32, tag="mm", name="mm")
            for c in range(2):
                nc.tensor.matmul(ps[:, c * C5:(c + 1) * C5],
                                 Gsel[:, j * 128:(j + 1) * 128], gET[:],
                                 start=True, stop=True)
            g = sgl.tile([128, R], F32, tag=f"gEp{j}", name=f"gEp{j}")
            nc.scalar.copy(g[:], ps[:])
            gEp.append(g)

        # ---- state: h0 = x0 @ Wh + bh, z0 = x0 @ Wz + bz ----
        h = wrk.tile([HID, R], F32, tag="h", name="h", bufs=2)
        z = wrk.tile([HID, R], F32, tag="z", name="z", bufs=2)
        h0p = pA.tile([HID, R], F32, tag="mm", name="mm")
        mm2(h0p, Wh, x0)
        nc.vector.tensor_scalar_add(h[:], h0p[:], bh[:])
        z0p = pA.tile([HID, R], F32, tag="mm", name="mm")
        mm2(z0p, Wz, x0)
        nc.vector.tensor_scalar_add(z[:], z0p[:], bz[:])

        def vfield(s4, hs, zs, kh, kz):
            u = 3 * (s4 // 4) + (s4 % 4)
            dxs = dxp.tile([IN, R], BF16, tag="dxs", name="dxs")
            nc.sync.dma_start(out=dxs[:], in_=d_pc[2 * u:2 * u + 2, :])
            # f path: two relu MLP layers + tanh head (i-major columns)
            x1p = pA.tile([HH, R], F32, tag="mm", name="mm")
            mm2(x1p, fw1, hs)
            x1 = wrk.tile([HH, R], F32, tag="fx", name="fx", bufs=2)
            act(x1[:], x1p[:], AF.Relu, bias=fb1[:])
            x2p = pA.tile([HH, R], F32, tag="mm", name="mm")
            mm2(x2p, fw2, x1)
            x2 = wrk.tile([HH, R], F32, tag="fx", name="fx", bufs=2)
            act(x2[:], x2p[:], AF.Relu, bias=fb2[:])
            vfp = pA.tile([HID * IN, R], F32, tag="mm", name="mm")
            mm2(vfp, fw3, x2)
            vf = wrk.tile([HID * IN, R], F32, tag="vf", name="vf")
            act(vf[:], vfp[:], AF.Tanh, bias=fb3[:])
            # dh = sum_i vf_i * dX_i  (dX broadcast via Bc, reduce via S3)
            dXb = pA.tile([IN * HID, R], F32, tag="mm", name="mm")
            mm2(dXb, Bc, dxs)
            nc.vector.tensor_mul(vf[:], vf[:], dXb[:])
            dhp = pB.tile([HID, R], F32, tag="acc", name="acc")
            mm2(dhp, S3, vf)
            nc.scalar.copy(kh[:], dhp[:])
            drp = pA.tile([128, R], F32, tag="mm", name="mm")
            mm2(drp, Erep, kh)
            dhrep = big1.tile([128, R], F32, tag="dhrep", name="dhrep")
            nc.scalar.copy(dhrep[:], drp[:])
            # g path: relu layer (feature-major), node-major transposes
            x1gp = pA.tile([HH, R], F32, tag="mm", name="mm")
            mm2(x1gp, gw1, zs)
            x1g = wrk.tile([HH, R], F32, tag="x1g", name="x1g")
            act(x1g[:], x1gp[:], AF.Relu, bias=gb1[:])
            xT = []
            for k_ in range(4):
                xtp = pT.tile([128, 2 * HH], F32, tag="pt", name="pt")
                for b_ in range(2):
                    nc.tensor.transpose(
                        xtp[:, b_ * HH:(b_ + 1) * HH],
                        x1g[:, b_ * N + k_ * 128: b_ * N + (k_ + 1) * 128],
                        ident[:HH, :HH])
                xt = wrk.tile([128, 2 * HH], F32, tag=f"xT{k_}",
                              name=f"xT{k_}")
                nc.vector.tensor_copy(xt[:], xtp[:])
                xT.append(xt)
            # graph conv: xg1 = A @ x1g per batch, recip folded in
            xg1n = []
            for i in range(4):
                xgp = pT.tile([128, 2 * HH], F32, tag="pt", name="pt")
                for k_ in range(4):
                    nc.tensor.matmul(xgp[:],
                                     expG[k_][:, i * 128:(i + 1) * 128],
                                     xT[k_][:],
                                     start=(k_ == 0), stop=(k_ == 3))
                xn = wrk.tile([128, 2 * HH], F32, tag=f"xg1n{i}",
                              name=f"xg1n{i}")
                nc.vector.tensor_scalar_mul(xn[:], xgp[:], recip[:, i:i + 1])
                xg1n.append(xn)
            xg1f = wrk.tile([HH, R], F32, tag="xg1f", name="xg1f")
            for i in range(4):
                for b_ in range(2):
                    btp = pT.tile([HH, 128], F32, tag="pt", name="pt")
                    nc.tensor.transpose(btp[:],
                                        xg1n[i][:, b_ * HH:(b_ + 1) * HH],
                                        ident[:, :])
                    nc.scalar.copy(
                        xg1f[:, b_ * N + i * 128: b_ * N + (i + 1) * 128],
                        btp[:])
            # per-node pooled weights: y = Wp^T xg scaled by gE_part,
            # reduced over EMB via S2 into x2g (abf preloaded via I32)
            x2gp = pB.tile([HH, R], F32, tag="acc", name="acc")
            for c in range(2):
                nc.tensor.matmul(x2gp[:, c * C5:(c + 1) * C5], I32[:],
                                 abT[:],
                                 start=True, stop=False, skip_group_check=True)
            for j in range(4):
                yp = pA.tile([128, R], F32, tag="mm", name="mm")
                for c in range(2):
                    sl = slice(c * C5, (c + 1) * C5)
                    nc.tensor.matmul(yp[:, sl], wpk0[:, j * 128:(j + 1) * 128],
                                     x1g[:, sl], start=True, stop=False)
                    nc.tensor.matmul(yp[:, sl], wpk1[:, j * 128:(j + 1) * 128],
                                     xg1f[:, sl], start=False, stop=True)
                t_ = big1.tile([128, R], F32, tag="ty", name="ty", bufs=2)
                nc.vector.tensor_mul(t_[:], yp[:], gEp[j][:])
                for c in range(2):
                    sl = slice(c * C5, (c + 1) * C5)
                    nc.tensor.matmul(x2gp[:, sl], S2[:], t_[:, sl],
                                     start=False, stop=(j == 3),
                                     skip_group_check=True)
            x2g = wrk.tile([HH, R], F32, tag="x2g", name="x2g")
            nc.scalar.copy(x2g[:], x2gp[:])
            # vg chunks; dz = sum vg_ho * dh_o accumulated via Sdz
            dzp = pB.tile([HID, R], F32, tag="acc", name="acc")
            for j in range(8):
                vgp = pA.tile([128, R], F32, tag="mm", name="mm")
                mm2(vgp, gwo[:, j * 128:(j + 1) * 128], x2g)
                vg = big2.tile([128, R], F32, tag="vg", name="vg")
                act(vg[:], vgp[:], AF.Tanh, bias=gbo[:, j:j + 1])
                nc.vector.tensor_mul(vg[:], vg[:], dhrep[:])
                for c in range(2):
                    sl = slice(c * C5, (c + 1) * C5)
                    nc.tensor.matmul(dzp[:, sl],
                                     Sdz[:, j * HID:(j + 1) * HID],
                                     vg[:, sl],
                                     start=(j == 0), stop=(j == 7),
                                     skip_group_check=True)
            nc.scalar.copy(kz[:], dzp[:])

        TT = nc.vector.tensor_tensor
        STT = nc.vector.scalar_tensor_tensor

        # RK4 with 3/8 rule, dt = 1 (times are arange; asserted on host)
        for s in range(nstep):
            kh = [wrk.tile([HID, R], F32, tag=f"kh{st}", name=f"kh{st}")
                  for st in range(4)]
            kz = [wrk.tile([HID, R], F32, tag=f"kz{st}", name=f"kz{st}")
                  for st in range(4)]
            vfield(4 * s + 0, h, z, kh[0], kz[0])
            hs = wrk.tile([HID, R], F32, tag="hs", name="hs", bufs=2)
            zs = wrk.tile([HID, R], F32, tag="zs", name="zs", bufs=2)
            STT(hs[:], kh[0][:], 1.0 / 3.0, h[:], op0=ALU.mult, op1=ALU.add)
            STT(zs[:], kz[0][:], 1.0 / 3.0, z[:], op0=ALU.mult, op1=ALU.add)
            vfield(4 * s + 1, hs, zs, kh[1], kz[1])
            hs2 = wrk.tile([HID, R], F32, tag="hs", name="hs", bufs=2)
            zs2 = wrk.tile([HID, R], F32, tag="zs", name="zs", bufs=2)
            STT(hs2[:], kh[0][:], -1.0 / 3.0, kh[1][:],
                op0=ALU.mult, op1=ALU.add)
            TT(hs2[:], hs2[:], h[:], op=ALU.add)
            STT(zs2[:], kz[0][:], -1.0 / 3.0, kz[1][:],
                op0=ALU.mult, op1=ALU.add)
            TT(zs2[:], zs2[:], z[:], op=ALU.add)
            vfield(4 * s + 2, hs2, zs2, kh[2], kz[2])
            hs3 = wrk.tile([HID, R], F32, tag="hs", name="hs", bufs=2)
            zs3 = wrk.tile([HID, R], F32, tag="zs", name="zs", bufs=2)
            STT(hs3[:], kh[1][:], -1.0, kh[0][:], op0=ALU.mult, op1=ALU.add)
            TT(hs3[:], hs3[:], kh[2][:], op=ALU.add)
            TT(hs3[:], hs3[:], h[:], op=ALU.add)
            STT(zs3[:], kz[1][:], -1.0, kz[0][:], op0=ALU.mult, op1=ALU.add)
            TT(zs3[:], zs3[:], kz[2][:], op=ALU.add)
            TT(zs3[:], zs3[:], z[:], op=ALU.add)
            vfield(4 * s + 3, hs3, zs3, kh[3], kz[3])
            hn = wrk.tile([HID, R], F32, tag="h", name="h", bufs=2)
            zn = wrk.tile([HID, R], F32, tag="z", name="z", bufs=2)
            TT(kh[1][:], kh[1][:], kh[2][:], op=ALU.add)
            STT(kh[1][:], kh[1][:], 3.0, kh[0][:], op0=ALU.mult, op1=ALU.add)
            TT(kh[1][:], kh[1][:], kh[3][:], op=ALU.add)
            STT(hn[:], kh[1][:], 0.125, h[:], op0=ALU.mult, op1=ALU.add)
            TT(kz[1][:], kz[1][:], kz[2][:], op=ALU.add)
            STT(kz[1][:], kz[1][:], 3.0, kz[0][:], op0=ALU.mult, op1=ALU.add)
            TT(kz[1][:], kz[1][:], kz[3][:], op=ALU.add)
            STT(zn[:], kz[1][:], 0.125, z[:], op0=ALU.mult, op1=ALU.add)
            h, z = hn, zn

        # ---- end conv ----
        op = pB.tile([OUT, R], F32, tag="acc", name="acc")
        mm2(op, cw, z)
        ob = wrk.tile([OUT, R], mybir.dt.bfloat16, tag="ob", name="ob")
        nc.vector.tensor_scalar_add(ob[:], op[:], cb[:])
        nc.sync.dma_start(out=d_out[:], in_=ob[:])

    _spill_excess_waits(nc)
    return nc


# ------------------------------------------------------------------
# host-side preprocessing
# ------------------------------------------------------------------
def host_inputs(a, nstep=NSTEP, overlap_put=None):
    """Build the global (concatenated-over-cores) input blobs.

    With overlap_put, each blob is async-transferred to the devices the
    moment it is ready, overlapping transfer with the remaining prep.
    """
    gE = a["gE"]
    times = a["times"]
    assert np.allclose(np.diff(times), 1.0, atol=1e-5), "RK dt=1 baked in"
    put = overlap_put if overlap_put is not None else (lambda x: x)

    # ---- weight blob (sharded 1/8th per core, AllGathered on device) ----
    fw3 = np.empty((HH, HID * IN), np.float32)
    fb3 = np.empty((HID * IN, 1), np.float32)
    for h_ in range(HID):
        for i in range(IN):
            fw3[:, i * HID + h_] = a["fWout"][:, h_ * IN + i]
            fb3[i * HID + h_, 0] = a["fbout"][h_ * IN + i]
    wpk = np.ascontiguousarray(
        np.transpose(a["gWpool"], (1, 2, 0, 3)).reshape(K, HH, EMB * HH))
    G = np.maximum(gE @ gE.T, 0.0).astype(np.float32)
    rs = np.exp(G).sum(axis=1)
    recip = np.ascontiguousarray((1.0 / rs).reshape(4, 128).T)
    vals = {
        "recip": recip, "gET": gE.T, "gbpool": a["gbpool"],
        "Wh": a["Wh"], "bh": a["bh"].reshape(-1, 1),
        "Wz": a["Wz"], "bz": a["bz"].reshape(-1, 1),
        "fw1": a["fWin"], "fb1": a["fbin"].reshape(-1, 1),
        "fw2": a["fWmid"], "fb2": a["fbmid"].reshape(-1, 1),
        "fw3": fw3, "fb3": fb3,
        "gw1": a["gWin"], "gb1": a["gbin"].reshape(-1, 1),
        "wpk0": wpk[0], "wpk1": wpk[1],
        "gwo": a["gWout"],
        "gbo": np.ascontiguousarray(a["gbout"].reshape(8, 128).T),
        "cw": np.ascontiguousarray(a["convW"].T),
        "cb": a["convb"].reshape(-1, 1),
    }
    wb = np.concatenate(
        [np.ascontiguousarray(vals[nm]).astype(np.float32).ravel()
         for nm, _ in _WSPEC]
        + [np.zeros(_WPAD - _WSIZE, np.float32)])
    out = {"wb": put(wb)}

    # ---- x0 (fp32, small) ----
    x0 = a["coeff_a"][:, :, 0, :]                   # (B, N, IN)
    x0g = np.ascontiguousarray(
        x0.reshape(NCORES, BS, N, IN).transpose(0, 3, 1, 2).reshape(
            NCORES * IN, R)).astype(np.float32)
    out["x0"] = put(x0g)

    # ---- spline derivatives at unique stage times (bf16) ----
    maxlen = T - 2
    ts_list = [float(times[0])]
    for s in range(nstep):
        t0, t1 = float(times[s]), float(times[s + 1])
        dt = t1 - t0
        ts_list += [t0 + dt / 3.0, t0 + 2.0 * dt / 3.0, t1]
    nu = len(ts_list)
    idxs = np.array([int(np.clip(np.sum(np.float32(t_) > times) - 1,
                                 0, maxlen)) for t_ in ts_list])
    fracs = np.array([np.float32(t_ - times[ix])
                      for t_, ix in zip(ts_list, idxs)],
                     np.float32).reshape(1, 1, nu, 1)
    dX = (a["coeff_b"][:, :, idxs]
          + (a["coeff_c2"][:, :, idxs]
             + a["coeff_d3"][:, :, idxs] * fracs) * fracs)  # (B, N, nu, IN)
    pcg = np.ascontiguousarray(
        dX.reshape(NCORES, BS, N, nu, IN).transpose(0, 3, 4, 1, 2).reshape(
            NCORES * nu * IN, R)).astype(ml_dtypes.bfloat16)
    out["pc"] = put(pcg)
    return out


_STATE = {}


def _get_nc():
    if "nc" not in _STATE:
        _STATE["nc"] = build_nc()
    return _STATE["nc"]


def _get_runner():
    """Cached jit(shard_map(bass_exec)) callable — built once so per-call
    cost is dispatch only (run_bass_kernel_spmd re-traces every call)."""
    if "runner" in _STATE:
        return _STATE["runner"]
    import jax
    from jax.sharding import Mesh, PartitionSpec
    from jax.experimental.shard_map import shard_map
    from concourse import bass2jax as b2j

    b2j.install_neuronx_cc_hook()
    nc = _get_nc()
    assert nc.dbg_addr is None
    partition_name = (nc.partition_id_tensor.name
                      if nc.partition_id_tensor else None)
    in_names, out_names, out_avals, zero_outs = [], [], [], []
    for alloc in nc.m.functions[0].allocations:
        if not isinstance(alloc, mybir.MemoryLocationSet):
            continue
        name = alloc.memorylocations[0].name
        if alloc.kind == "ExternalInput":
            if name != partition_name:
                in_names.append(name)
        elif alloc.kind == "ExternalOutput":
            shape = tuple(alloc.tensor_shape)
            dtype = mybir.dt.np(alloc.dtype)
            out_names.append(name)
            out_avals.append(jax.core.ShapedArray(shape, dtype))
            zero_outs.append(np.zeros((NCORES * shape[0], *shape[1:]), dtype))
    n_params = len(in_names)
    all_names = list(in_names) + list(out_names)
    if partition_name is not None:
        all_names.append(partition_name)

    def _body(*args):
        operands = list(args)
        if partition_name is not None:
            operands.append(b2j.partition_id_tensor())
        outs = b2j._bass_exec_p.bind(
            *operands,
            out_avals=tuple(out_avals),
            in_names=tuple(all_names),
            out_names=tuple(out_names),
            lowering_input_output_aliases=(),
            sim_require_finite=True,
            sim_require_nnan=True,
            nc=nc,
        )
        return tuple(outs)

    devices = jax.devices()[:NCORES]
    mesh = Mesh(np.asarray(devices), ("core",))
    n_outs = len(out_names)
    sharded = jax.jit(
        shard_map(_body, mesh=mesh,
                  in_specs=(PartitionSpec("core"),) * (n_params + n_outs),
                  out_specs=(PartitionSpec("core"),) * n_outs,
                  check_rep=False),
        donate_argnums=tuple(range(n_params, n_params + n_outs)),
        keep_unused=True,
    )

    from jax.sharding import NamedSharding
    shardspec = NamedSharding(mesh, PartitionSpec("core"))

    def _put_zeros():
        return [jax.device_put(z, shardspec) for z in zero_outs]

    def run_prepacked(by_name):
        concat_in = [by_name[nm] for nm in in_names]
        zs = _STATE.pop("zeros_dev", None)
        if zs is None:
            zs = _put_zeros()
        out_arrs = sharded(*concat_in, *zs)
        res = [
            {nm: np.asarray(out_arrs[i]).reshape(
                NCORES, *out_avals[i].shape)[c]
             for i, nm in enumerate(out_names)}
            for c in range(NCORES)
        ]
        # replenish asynchronously; only the enqueue is paid here
        _STATE["zeros_dev"] = [jax.device_put(z, shardspec)
                               for z in zero_outs]
        return res

    class _Runner:
        prepacked = staticmethod(run_prepacked)
        put = staticmethod(lambda arr: jax.device_put(arr, shardspec))

    _STATE["runner"] = _Runner
    return _Runner


def _warm():
    """Trigger neuron compile + PJRT executable load with dummy inputs."""
    try:
        nc = _get_nc()
        a = {}
        a["times"] = np.arange(T, dtype=np.float32)
        for nm, sh in [("coeff_a", (B, N, T - 1, IN)),
                       ("coeff_b", (B, N, T - 1, IN)),
                       ("coeff_c2", (B, N, T - 1, IN)),
                       ("coeff_d3", (B, N, T - 1, IN)),
                       ("Wh", (IN, HID)), ("bh", (HID,)),
                       ("Wz", (IN, HID)), ("bz", (HID,)),
                       ("fWin", (HID, HH)), ("fbin", (HH,)),
                       ("fWmid", (HH, HH)), ("fbmid", (HH,)),
                       ("fWout", (HH, HID * IN)), ("fbout", (HID * IN,)),
                       ("gWin", (HID, HH)), ("gbin", (HH,)),
                       ("gE", (N, EMB)), ("gWpool", (EMB, K, HH, HH)),
                       ("gbpool", (EMB, HH)), ("gWout", (HH, HID * HID)),
                       ("gbout", (HID * HID,)), ("convW", (OUT, HID)),
                       ("convb", (OUT,))]:
            a[nm] = np.zeros(sh, np.float32)
        run = _get_runner()
        for _ in range(2):
            run.prepacked(host_inputs(a, overlap_put=run.put))
        _STATE["warm"] = True
    except Exception as e:  # pragma: no cover - keep import usable
        import traceback
        traceback.print_exc()
        _STATE["warm_err"] = e


def kernel(**inputs):
    run = _get_runner()
    a = {k_: np.asarray(v, dtype=np.float32) for k_, v in inputs.items()}
    results = run.prepacked(host_inputs(a, overlap_put=run.put))
    full = np.empty((B, 1, N, OUT), np.float32)
    for c in range(NCORES):
        o = np.asarray(results[c]["out"]).astype(np.float32)  # (OUT, R)
        full[c * BS:(c + 1) * BS, 0] = (
            o.reshape(OUT, BS, N).transpose(1, 2, 0))
    return full


_warm()


# revision 19
# speedup vs baseline: 1.2366x; 1.2366x over previous
"""NeuralGCDE on 8 NeuronCores: full RK4 ODE integration on device.

Sharding: data-parallel over batch B=16 -> 2 batch elements per core
(rows r = b*N + n, R = 1024 per core). All graph/MLP params replicated.

Device layout is feature-major (features on SBUF partitions, rows on the
free dim). Every contraction is a PE matmul; partition-dim reductions and
broadcasts use structured 0/1 matrices as stationary operands. The
softmax adjacency (exp(relu(gE gE^T)) with row scaling) is built on
device; only the row-sum reciprocals (512 floats) come from host.

All one-time work (bass build, neuron compile, PJRT load) happens at
import; kernel(**inputs) does host repacks + one SPMD dispatch.
"""
import ml_dtypes
import numpy as np

import concourse.bass as bass
import concourse.mybir as mybir
import concourse.tile as tile

B, N, T = 16, 512, 12
IN, HID, HH, EMB, K, OUT = 2, 32, 32, 16, 2, 12
NCORES = 8
BS = B // NCORES            # 2
R = BS * N                  # 1024
NSTEP = T - 1               # 11
F32 = mybir.dt.float32
AF = mybir.ActivationFunctionType
ALU = mybir.AluOpType

# (name, shape) of every shared parameter, packed flat into one upload
_WSPEC = [
    ("recip", (128, 4)), ("gET", (EMB, N)), ("gbpool", (EMB, HH)),
    ("Wh", (IN, HID)), ("bh", (HID, 1)), ("Wz", (IN, HID)), ("bz", (HID, 1)),
    ("fw1", (HID, HH)), ("fb1", (HH, 1)), ("fw2", (HH, HH)), ("fb2", (HH, 1)),
    ("fw3", (HH, HID * IN)), ("fb3", (HID * IN, 1)),
    ("gw1", (HID, HH)), ("gb1", (HH, 1)),
    ("gbo", (128, 8)),
    ("cw", (HID, OUT)), ("cb", (OUT, 1)),
]
_WSIZE = sum(int(np.prod(s)) for _, s in _WSPEC)
_WPAD = ((_WSIZE + 7) // 8) * 8
_WSH = _WPAD // 8
# large weights, uploaded bf16 (cast to fp32 on device at setup)
_W2SPEC = [
    ("wpk0", (HH, EMB * HH)), ("wpk1", (HH, EMB * HH)),
    ("gwo", (HH, HID * HID)),
]
_W2SIZE = sum(int(np.prod(s)) for _, s in _W2SPEC)
_W2PAD = ((_W2SIZE + 7) // 8) * 8
_W2SH = _W2PAD // 8

_NO_SPILL = {"InstEventSemaphore", "InstUnconditionalBranch",
             "InstConditionalBranch"}


def _spill_excess_waits(nc):
    """Walrus ISA structs hold one sync-wait slot on most instructions.
    Tile can emit several. Move excess waits onto InstEventSemaphore
    carriers inserted just before, on the same engine (waiting earlier on
    the same engine stream is always sound)."""
    nspill = 0
    for f in nc.m.functions:
        for blk in f.blocks:
            lst = blk.instructions
            i = 0
            while i < len(lst):
                ins = lst[i]
                si = ins.sync_info
                if (type(ins).__name__ in _NO_SPILL or si is None
                        or not si.on_wait or len(si.on_wait) <= 1):
                    i += 1
                    continue
                waits = list(si.on_wait)
                keep, excess = waits[-1:], waits[:-1]
                ins.sync_info = mybir.SyncInfo(on_wait=keep,
                                               on_update=list(si.on_update))
                carriers = []
                while excess:
                    chunk, excess = excess[:2], excess[2:]
                    es = mybir.InstEventSemaphore(
                        name=f"Wspill-{nspill}", ins=[], outs=[])
                    nspill += 1
                    es.engine = ins.engine
                    es.sync_info = mybir.SyncInfo(on_wait=chunk, on_update=[])
                    carriers.append(es)
                for k_, es in enumerate(carriers):
                    lst.insert(i + k_, es)
                i += len(carriers) + 1
    return nspill


def build_nc(nstep=NSTEP):
    nc = bass.Bass()

    def dp(name, shape, out=False):
        return nc.declare_dram_parameter(name, list(shape), F32, isOutput=out)

    NU = 3 * nstep + 1                     # unique dX stage rows
    BF16 = mybir.dt.bfloat16
    d_pc = nc.declare_dram_parameter("pc", [NU * IN, R], BF16, isOutput=False)
    d_x0 = nc.declare_dram_parameter("x0", [IN, R], BF16, isOutput=False)
    d_wb = nc.declare_dram_parameter("wb", [_WSH], F32, isOutput=False)
    wb_in = nc.dram_tensor("wb_in", [_WSH], F32)
    wb_all = nc.dram_tensor("wb_all", [_WPAD], F32)
    d_wb2 = nc.declare_dram_parameter("wb2", [_W2SH], BF16, isOutput=False)
    wb2_in = nc.dram_tensor("wb2_in", [_W2SH], BF16)
    wb2_all = nc.dram_tensor("wb2_all", [_W2PAD], BF16)
    d_out = nc.declare_dram_parameter("out", [OUT, R], mybir.dt.bfloat16,
                                      isOutput=True)

    C5 = 512  # fp32 moving-operand free-dim limit

    from contextlib import ExitStack
    with ExitStack() as es:
        tc = es.enter_context(tile.TileContext(nc))
        sgl = es.enter_context(tc.tile_pool(name="sgl", bufs=1))
        wrk = es.enter_context(tc.tile_pool(name="wrk", bufs=1))
        big1 = es.enter_context(tc.tile_pool(name="big1", bufs=1))
        big2 = es.enter_context(tc.tile_pool(name="big2", bufs=2))
        dxp = es.enter_context(tc.tile_pool(name="dxp", bufs=2))
        pA = es.enter_context(tc.tile_pool(name="pA", bufs=2, space="PSUM"))
        pB = es.enter_context(tc.tile_pool(name="pB", bufs=1, space="PSUM"))
        pT = es.enter_context(tc.tile_pool(name="pT", bufs=2, space="PSUM"))

        # gather the weight blobs: each core uploaded 1/8th
        nc.sync.dma_start(out=wb_in[:], in_=d_wb[:])
        nc.gpsimd.collective_compute(
            "AllGather", ALU.bypass,
            replica_groups=[list(range(NCORES))],
            ins=[wb_in[:]], outs=[wb_all[:]])
        nc.sync.dma_start(out=wb2_in[:], in_=d_wb2[:])
        nc.gpsimd.collective_compute(
            "AllGather", ALU.bypass,
            replica_groups=[list(range(NCORES))],
            ins=[wb2_in[:]], outs=[wb2_all[:]])

        woff = [0]

        def load(name, shape):
            p_, f_ = shape
            t = sgl.tile([p_, f_], F32, tag=name, name=name)
            nc.sync.dma_start(
                out=t[:],
                in_=wb_all[woff[0]:woff[0] + p_ * f_].rearrange(
                    "(p f) -> p f", p=p_))
            woff[0] += p_ * f_
            return t

        W = {nm: load(nm, sh) for nm, sh in _WSPEC}
        (recip, gET, gbpool, Wh, bh, Wz, bz, fw1, fb1, fw2, fb2, fw3, fb3,
         gw1, gb1, gbo, cw, cb) = (W[nm] for nm, _ in _WSPEC)

        w2off = [0]

        def load2(name, shape):
            p_, f_ = shape
            tb = sgl.tile([p_, f_], BF16, tag=name + "b", name=name + "b")
            nc.sync.dma_start(
                out=tb[:],
                in_=wb2_all[w2off[0]:w2off[0] + p_ * f_].rearrange(
                    "(p f) -> p f", p=p_))
            w2off[0] += p_ * f_
            t = sgl.tile([p_, f_], F32, tag=name, name=name)
            nc.scalar.copy(t[:], tb[:])
            return t

        W2 = {nm: load2(nm, sh) for nm, sh in _W2SPEC}
        wpk0, wpk1, gwo = (W2[nm] for nm, _ in _W2SPEC)
        x0b = sgl.tile([IN, R], BF16, tag="x0b", name="x0b")
        nc.sync.dma_start(out=x0b[:], in_=d_x0[:])
        x0 = sgl.tile([IN, R], F32, tag="x0", name="x0")
        nc.scalar.copy(x0[:], x0b[:])

        # ---- structured 0/1 matrices, built in place ----
        NE = ALU.not_equal

        def zeros_tile(name, shape):
            t = sgl.tile(list(shape), F32, tag=name, name=name)
            nc.gpsimd.memset(t[:], 0.0)
            return t

        def aff(t, ap, pattern, base=0, cm=0):
            nc.gpsimd.affine_select(out=ap, in_=ap, compare_op=NE, fill=1.0,
                                    base=base, pattern=pattern,
                                    channel_multiplier=cm)

        ident = zeros_tile("ident", (128, 128))
        aff(ident, ident[:], [[-1, 128]], cm=1)
        I32 = zeros_tile("I32", (HH, HH))
        aff(I32, I32[:], [[-1, HH]], cm=1)
        Bc = sgl.tile([IN, IN * HID], BF16, tag="Bc", name="Bc")
        nc.gpsimd.memset(Bc[:], 0.0)
        aff(Bc, Bc[:].rearrange("p (j y) -> p j y", y=HID), [[-1, IN], [0, HID]],
            cm=1)
        Erep = zeros_tile("Erep", (HID, 128))      # 1 iff col%32 == p
        aff(Erep, Erep[:].rearrange("p (j y) -> p j y", y=HID),
            [[0, 4], [-1, HID]], cm=1)
        S3 = zeros_tile("S3", (IN * HID, HID))     # 1 iff p%32 == col
        aff(S3, S3[:], [[-1, HID]], cm=1)
        aff(S3, S3[:], [[-1, HID]], base=-HID, cm=1)
        S2 = zeros_tile("S2", (128, HH))           # 1 iff p%32 == col
        for q in range(4):
            aff(S2, S2[:], [[-1, HH]], base=-q * HH, cm=1)
        Gsel = zeros_tile("Gsel", (EMB, 4 * 128))  # 1 iff col//32 == p
        aff(Gsel, Gsel[:].rearrange("p (j y) -> p j y", y=32),
            [[-1, EMB], [0, 32]], cm=1)
        # Sdz[p, j*32+y] = 1 iff y == 4j + p//32, composed as E4.T @ Cdz
        E4 = zeros_tile("E4", (4, 128))            # 1 iff col//32 == p
        aff(E4, E4[:].rearrange("p (j y) -> p j y", y=32), [[-1, 4], [0, 32]],
            cm=1)
        Cdz = zeros_tile("Cdz", (4, 8 * HID))      # 1 iff y == 4j + p
        aff(Cdz, Cdz[:].rearrange("p (j y) -> p j y", y=HID),
            [[4, 8], [-1, HID]], cm=1)
        sdzp = pA.tile([128, 8 * HID], F32, tag="mm", name="mm")
        nc.tensor.matmul(sdzp[:], E4[:], Cdz[:], start=True, stop=True)
        Sdz = sgl.tile([128, 8 * HID], F32, tag="Sdz", name="Sdz")
        nc.scalar.copy(Sdz[:], sdzp[:])

        # ---- abT[o, n] = (gE @ gbpool).T, used for both batch halves ----
        abp = pA.tile([HH, N], F32, tag="mm", name="mm")
        nc.tensor.matmul(abp[:], gbpool[:], gET[:], start=True, stop=True)
        abT = sgl.tile([HH, N], F32, tag="abT", name="abT")
        nc.scalar.copy(abT[:], abp[:])

        def mm2(ps, lhsT, rhs, start=True, stop=True):
            for c in range(2):
                nc.tensor.matmul(ps[:, c * C5:(c + 1) * C5], lhsT,
                                 rhs[:, c * C5:(c + 1) * C5],
                                 start=start, stop=stop)

        def act(out, in_, func, bias=0.0):
            nc.scalar.activation(out, in_, func, bias=bias)

        # ---- adjacency: expG chunks (exp(relu(gE gE^T)), m-major) ----
        expG = []
        for i in range(4):
            gp = pA.tile([128, N], F32, tag="mm", name="mm")
            nc.tensor.matmul(gp[:], gET[:, i * 128:(i + 1) * 128], gET[:],
                             start=True, stop=True)
            eg = sgl.tile([128, N], F32, tag=f"expG{i}", name=f"expG{i}")
            act(eg[:], gp[:], AF.Relu)
            act(eg[:], eg[:], AF.Exp)
            expG.append(eg)

        # ---- gE_part chunks: gEp_j[p, r] = gE[n(r), (j*128+p)//32] ----
        gEp = []
        for j in range(4):
            ps = pA.tile([128, R], F32, tag="mm", name="mm")
            for c in range(2):
                nc.tensor.matmul(ps[:, c * C5:(c + 1) * C5],
                                 Gsel[:, j * 128:(j + 1) * 128], gET[:],
                                 start=True, stop=True)
            g = sgl.tile([128, R], F32, tag=f"gEp{j}", name=f"gEp{j}")
            nc.scalar.copy(g[:], ps[:])
            gEp.append(g)

        # ---- state: h0 = x0 @ Wh + bh, z0 = x0 @ Wz + bz ----
        h = wrk.tile([HID, R], F32, tag="h", name="h", bufs=2)
        z = wrk.tile([HID, R], F32, tag="z", name="z", bufs=2)
        h0p = pA.tile([HID, R], F32, tag="mm", name="mm")
        mm2(h0p, Wh, x0)
        nc.vector.tensor_scalar_add(h[:], h0p[:], bh[:])
        z0p = pA.tile([HID, R], F32, tag="mm", name="mm")
        mm2(z0p, Wz, x0)
        nc.vector.tensor_scalar_add(z[:], z0p[:], bz[:])

        def vfield(s4, hs, zs, kh, kz):
            u = 3 * (s4 // 4) + (s4 % 4)
            dxs = dxp.tile([IN, R], BF16, tag="dxs", name="dxs")
            nc.sync.dma_start(out=dxs[:], in_=d_pc[2 * u:2 * u + 2, :])
            # f path: two relu MLP layers + tanh head (i-major columns)
            x1p = pA.tile([HH, R], F32, tag="mm", name="mm")
            mm2(x1p, fw1, hs)
            x1 = wrk.tile([HH, R], F32, tag="fx", name="fx", bufs=2)
            act(x1[:], x1p[:], AF.Relu, bias=fb1[:])
            x2p = pA.tile([HH, R], F32, tag="mm", name="mm")
            mm2(x2p, fw2, x1)
            x2 = wrk.tile([HH, R], F32, tag="fx", name="fx", bufs=2)
            act(x2[:], x2p[:], AF.Relu, bias=fb2[:])
            vfp = pA.tile([HID * IN, R], F32, tag="mm", name="mm")
            mm2(vfp, fw3, x2)
            vf = wrk.tile([HID * IN, R], F32, tag="vf", name="vf")
            act(vf[:], vfp[:], AF.Tanh, bias=fb3[:])
            # dh = sum_i vf_i * dX_i  (dX broadcast via Bc, reduce via S3)
            dXb = pA.tile([IN * HID, R], F32, tag="mm", name="mm")
            mm2(dXb, Bc, dxs)
            nc.vector.tensor_mul(vf[:], vf[:], dXb[:])
            dhp = pB.tile([HID, R], F32, tag="acc", name="acc")
            mm2(dhp, S3, vf)
            nc.scalar.copy(kh[:], dhp[:])
            drp = pA.tile([128, R], F32, tag="mm", name="mm")
            mm2(drp, Erep, kh)
            dhrep = big1.tile([128, R], F32, tag="dhrep", name="dhrep")
            nc.scalar.copy(dhrep[:], drp[:])
            # g path: relu layer (feature-major), node-major transposes
            x1gp = pA.tile([HH, R], F32, tag="mm", name="mm")
            mm2(x1gp, gw1, zs)
            x1g = wrk.tile([HH, R], F32, tag="x1g", name="x1g")
            act(x1g[:], x1gp[:], AF.Relu, bias=gb1[:])
            xT = []
            for k_ in range(4):
                xtp = pT.tile([128, 2 * HH], F32, tag="pt", name="pt")
                for b_ in range(2):
                    nc.tensor.transpose(
                        xtp[:, b_ * HH:(b_ + 1) * HH],
                        x1g[:, b_ * N + k_ * 128: b_ * N + (k_ + 1) * 128],
                        ident[:HH, :HH])
                xt = wrk.tile([128, 2 * HH], F32, tag=f"xT{k_}",
                              name=f"xT{k_}")
                nc.vector.tensor_copy(xt[:], xtp[:])
                xT.append(xt)
            # graph conv: xg1 = A @ x1g per batch, recip folded in
            xg1n = []
            for i in range(4):
                xgp = pT.tile([128, 2 * HH], F32, tag="pt", name="pt")
                for k_ in range(4):
                    nc.tensor.matmul(xgp[:],
                                     expG[k_][:, i * 128:(i + 1) * 128],
                                     xT[k_][:],
                                     start=(k_ == 0), stop=(k_ == 3))
                xn = wrk.tile([128, 2 * HH], F32, tag=f"xg1n{i}",
                              name=f"xg1n{i}")
                nc.vector.tensor_scalar_mul(xn[:], xgp[:], recip[:, i:i + 1])
                xg1n.append(xn)
            xg1f = wrk.tile([HH, R], F32, tag="xg1f", name="xg1f")
            for i in range(4):
                for b_ in range(2):
                    btp = pT.tile([HH, 128], F32, tag="pt", name="pt")
                    nc.tensor.transpose(btp[:],
                                        xg1n[i][:, b_ * HH:(b_ + 1) * HH],
                                        ident[:, :])
                    nc.scalar.copy(
                        xg1f[:, b_ * N + i * 128: b_ * N + (i + 1) * 128],
                        btp[:])
            # per-node pooled weights: y = Wp^T xg scaled by gE_part,
            # reduced over EMB via S2 into x2g (abf preloaded via I32)
            x2gp = pB.tile([HH, R], F32, tag="acc", name="acc")
            for c in range(2):
                nc.tensor.matmul(x2gp[:, c * C5:(c + 1) * C5], I32[:],
                                 abT[:],
                                 start=True, stop=False, skip_group_check=True)
            for j in range(4):
                yp = pA.tile([128, R], F32, tag="mm", name="mm")
                for c in range(2):
                    sl = slice(c * C5, (c + 1) * C5)
                    nc.tensor.matmul(yp[:, sl], wpk0[:, j * 128:(j + 1) * 128],
                                     x1g[:, sl], start=True, stop=False)
                    nc.tensor.matmul(yp[:, sl], wpk1[:, j * 128:(j + 1) * 128],
                                     xg1f[:, sl], start=False, stop=True)
                t_ = big1.tile([128, R], F32, tag="ty", name="ty", bufs=2)
                nc.vector.tensor_mul(t_[:], yp[:], gEp[j][:])
                for c in range(2):
                    sl = slice(c * C5, (c + 1) * C5)
                    nc.tensor.matmul(x2gp[:, sl], S2[:], t_[:, sl],
                                     start=False, stop=(j == 3),
                                     skip_group_check=True)
            x2g = wrk.tile([HH, R], F32, tag="x2g", name="x2g")
            nc.scalar.copy(x2g[:], x2gp[:])
            # vg chunks; dz = sum vg_ho * dh_o accumulated via Sdz
            dzp = pB.tile([HID, R], F32, tag="acc", name="acc")
            for j in range(8):
                vgp = pA.tile([128, R], F32, tag="mm", name="mm")
                mm2(vgp, gwo[:, j * 128:(j + 1) * 128], x2g)
                vg = big2.tile([128, R], F32, tag="vg", name="vg")
                act(vg[:], vgp[:], AF.Tanh, bias=gbo[:, j:j + 1])
                nc.vector.tensor_mul(vg[:], vg[:], dhrep[:])
                for c in range(2):
                    sl = slice(c * C5, (c + 1) * C5)
                    nc.tensor.matmul(dzp[:, sl],
                                     Sdz[:, j * HID:(j + 1) * HID],
                                     vg[:, sl],
                                     start=(j == 0), stop=(j == 7),
                                     skip_group_check=True)
            nc.scalar.copy(kz[:], dzp[:])

        TT = nc.vector.tensor_tensor
        STT = nc.vector.scalar_tensor_tensor

        # RK4 with 3/8 rule, dt = 1 (times are arange; asserted on host)
        for s in range(nstep):
            kh = [wrk.tile([HID, R], F32, tag=f"kh{st}", name=f"kh{st}")
                  for st in range(4)]
            kz = [wrk.tile([HID, R], F32, tag=f"kz{st}", name=f"kz{st}")
                  for st in range(4)]
            vfield(4 * s + 0, h, z, kh[0], kz[0])
            hs = wrk.tile([HID, R], F32, tag="hs", name="hs", bufs=2)
            zs = wrk.tile([HID, R], F32, tag="zs", name="zs", bufs=2)
            STT(hs[:], kh[0][:], 1.0 / 3.0, h[:], op0=ALU.mult, op1=ALU.add)
            STT(zs[:], kz[0][:], 1.0 / 3.0, z[:], op0=ALU.mult, op1=ALU.add)
            vfield(4 * s + 1, hs, zs, kh[1], kz[1])
            hs2 = wrk.tile([HID, R], F32, tag="hs", name="hs", bufs=2)
            zs2 = wrk.tile([HID, R], F32, tag="zs", name="zs", bufs=2)
            STT(hs2[:], kh[0][:], -1.0 / 3.0, kh[1][:],
                op0=ALU.mult, op1=ALU.add)
            TT(hs2[:], hs2[:], h[:], op=ALU.add)
            STT(zs2[:], kz[0][:], -1.0 / 3.0, kz[1][:],
                op0=ALU.mult, op1=ALU.add)
            TT(zs2[:], zs2[:], z[:], op=ALU.add)
            vfield(4 * s + 2, hs2, zs2, kh[2], kz[2])
            hs3 = wrk.tile([HID, R], F32, tag="hs", name="hs", bufs=2)
            zs3 = wrk.tile([HID, R], F32, tag="zs", name="zs", bufs=2)
            STT(hs3[:], kh[1][:], -1.0, kh[0][:], op0=ALU.mult, op1=ALU.add)
            TT(hs3[:], hs3[:], kh[2][:], op=ALU.add)
            TT(hs3[:], hs3[:], h[:], op=ALU.add)
            STT(zs3[:], kz[1][:], -1.0, kz[0][:], op0=ALU.mult, op1=ALU.add)
            TT(zs3[:], zs3[:], kz[2][:], op=ALU.add)
            TT(zs3[:], zs3[:], z[:], op=ALU.add)
            vfield(4 * s + 3, hs3, zs3, kh[3], kz[3])
            hn = wrk.tile([HID, R], F32, tag="h", name="h", bufs=2)
            zn = wrk.tile([HID, R], F32, tag="z", name="z", bufs=2)
            TT(kh[1][:], kh[1][:], kh[2][:], op=ALU.add)
            STT(kh[1][:], kh[1][:], 3.0, kh[0][:], op0=ALU.mult, op1=ALU.add)
            TT(kh[1][:], kh[1][:], kh[3][:], op=ALU.add)
            STT(hn[:], kh[1][:], 0.125, h[:], op0=ALU.mult, op1=ALU.add)
            TT(kz[1][:], kz[1][:], kz[2][:], op=ALU.add)
            STT(kz[1][:], kz[1][:], 3.0, kz[0][:], op0=ALU.mult, op1=ALU.add)
            TT(kz[1][:], kz[1][:], kz[3][:], op=ALU.add)
            STT(zn[:], kz[1][:], 0.125, z[:], op0=ALU.mult, op1=ALU.add)
            h, z = hn, zn

        # ---- end conv ----
        op = pB.tile([OUT, R], F32, tag="acc", name="acc")
        mm2(op, cw, z)
        ob = wrk.tile([OUT, R], mybir.dt.bfloat16, tag="ob", name="ob")
        nc.vector.tensor_scalar_add(ob[:], op[:], cb[:])
        nc.sync.dma_start(out=d_out[:], in_=ob[:])

    _spill_excess_waits(nc)
    return nc


# ------------------------------------------------------------------
# host-side preprocessing
# ------------------------------------------------------------------
def host_inputs(a, nstep=NSTEP, overlap_put=None):
    """Build the global (concatenated-over-cores) input blobs.

    With overlap_put, each blob is async-transferred to the devices the
    moment it is ready, overlapping transfer with the remaining prep.
    """
    gE = a["gE"]
    times = a["times"]
    assert np.allclose(np.diff(times), 1.0, atol=1e-5), "RK dt=1 baked in"
    put = overlap_put if overlap_put is not None else (lambda x: x)

    # ---- weight blob (sharded 1/8th per core, AllGathered on device) ----
    fw3 = np.empty((HH, HID * IN), np.float32)
    fb3 = np.empty((HID * IN, 1), np.float32)
    for h_ in range(HID):
        for i in range(IN):
            fw3[:, i * HID + h_] = a["fWout"][:, h_ * IN + i]
            fb3[i * HID + h_, 0] = a["fbout"][h_ * IN + i]
    wpk = np.ascontiguousarray(
        np.transpose(a["gWpool"], (1, 2, 0, 3)).reshape(K, HH, EMB * HH))
    G = np.maximum(gE @ gE.T, 0.0).astype(np.float32)
    rs = np.exp(G).sum(axis=1)
    recip = np.ascontiguousarray((1.0 / rs).reshape(4, 128).T)
    vals = {
        "wpk0": wpk[0], "wpk1": wpk[1], "gwo": a["gWout"],
        "recip": recip, "gET": gE.T, "gbpool": a["gbpool"],
        "Wh": a["Wh"], "bh": a["bh"].reshape(-1, 1),
        "Wz": a["Wz"], "bz": a["bz"].reshape(-1, 1),
        "fw1": a["fWin"], "fb1": a["fbin"].reshape(-1, 1),
        "fw2": a["fWmid"], "fb2": a["fbmid"].reshape(-1, 1),
        "fw3": fw3, "fb3": fb3,
        "gw1": a["gWin"], "gb1": a["gbin"].reshape(-1, 1),
        "gbo": np.ascontiguousarray(a["gbout"].reshape(8, 128).T),
        "cw": np.ascontiguousarray(a["convW"].T),
        "cb": a["convb"].reshape(-1, 1),
    }
    wb = np.concatenate(
        [np.ascontiguousarray(vals[nm]).astype(np.float32).ravel()
         for nm, _ in _WSPEC]
        + [np.zeros(_WPAD - _WSIZE, np.float32)])
    out = {"wb": put(wb)}
    wb2 = np.concatenate(
        [np.ascontiguousarray(vals[nm]).astype(ml_dtypes.bfloat16).ravel()
         for nm, _ in _W2SPEC]
        + [np.zeros(_W2PAD - _W2SIZE, ml_dtypes.bfloat16)])
    out["wb2"] = put(wb2)

    # ---- x0 (fp32, small) ----
    x0 = a["coeff_a"][:, :, 0, :]                   # (B, N, IN)
    x0g = np.ascontiguousarray(
        x0.reshape(NCORES, BS, N, IN).transpose(0, 3, 1, 2).reshape(
            NCORES * IN, R)).astype(ml_dtypes.bfloat16)
    out["x0"] = put(x0g)

    # ---- spline derivatives at unique stage times (bf16) ----
    maxlen = T - 2
    ts_list = [float(times[0])]
    for s in range(nstep):
        t0, t1 = float(times[s]), float(times[s + 1])
        dt = t1 - t0
        ts_list += [t0 + dt / 3.0, t0 + 2.0 * dt / 3.0, t1]
    nu = len(ts_list)
    idxs = np.array([int(np.clip(np.sum(np.float32(t_) > times) - 1,
                                 0, maxlen)) for t_ in ts_list])
    fracs = np.array([np.float32(t_ - times[ix])
                      for t_, ix in zip(ts_list, idxs)],
                     np.float32).reshape(1, 1, nu, 1)
    dX = (a["coeff_b"][:, :, idxs]
          + (a["coeff_c2"][:, :, idxs]
             + a["coeff_d3"][:, :, idxs] * fracs) * fracs)  # (B, N, nu, IN)
    pcg = np.ascontiguousarray(
        dX.reshape(NCORES, BS, N, nu, IN).transpose(0, 3, 4, 1, 2).reshape(
            NCORES * nu * IN, R)).astype(ml_dtypes.bfloat16)
    out["pc"] = put(pcg)
    return out


_STATE = {}


def _get_nc():
    if "nc" not in _STATE:
        _STATE["nc"] = build_nc()
    return _STATE["nc"]


def _get_runner():
    """Cached jit(shard_map(bass_exec)) callable — built once so per-call
    cost is dispatch only (run_bass_kernel_spmd re-traces every call)."""
    if "runner" in _STATE:
        return _STATE["runner"]
    import jax
    from jax.sharding import Mesh, PartitionSpec
    from jax.experimental.shard_map import shard_map
    from concourse import bass2jax as b2j

    b2j.install_neuronx_cc_hook()
    nc = _get_nc()
    assert nc.dbg_addr is None
    partition_name = (nc.partition_id_tensor.name
                      if nc.partition_id_tensor else None)
    in_names, out_names, out_avals, zero_outs = [], [], [], []
    for alloc in nc.m.functions[0].allocations:
        if not isinstance(alloc, mybir.MemoryLocationSet):
            continue
        name = alloc.memorylocations[0].name
        if alloc.kind == "ExternalInput":
            if name != partition_name:
                in_names.append(name)
        elif alloc.kind == "ExternalOutput":
            shape = tuple(alloc.tensor_shape)
            dtype = mybir.dt.np(alloc.dtype)
            out_names.append(name)
            out_avals.append(jax.core.ShapedArray(shape, dtype))
            zero_outs.append(np.zeros((NCORES * shape[0], *shape[1:]), dtype))
    n_params = len(in_names)
    all_names = list(in_names) + list(out_names)
    if partition_name is not None:
        all_names.append(partition_name)

    def _body(*args):
        operands = list(args)
        if partition_name is not None:
            operands.append(b2j.partition_id_tensor())
        outs = b2j._bass_exec_p.bind(
            *operands,
            out_avals=tuple(out_avals),
            in_names=tuple(all_names),
            out_names=tuple(out_names),
            lowering_input_output_aliases=(),
            sim_require_finite=True,
            sim_require_nnan=True,
            nc=nc,
        )
        return tuple(outs)

    devices = jax.devices()[:NCORES]
    mesh = Mesh(np.asarray(devices), ("core",))
    n_outs = len(out_names)
    sharded = jax.jit(
        shard_map(_body, mesh=mesh,
                  in_specs=(PartitionSpec("core"),) * (n_params + n_outs),
                  out_specs=(PartitionSpec("core"),) * n_outs,
                  check_rep=False),
        donate_argnums=tuple(range(n_params, n_params + n_outs)),
        keep_unused=True,
    )

    from jax.sharding import NamedSharding
    shardspec = NamedSharding(mesh, PartitionSpec("core"))

    def _put_zeros():
        return [jax.device_put(z, shardspec) for z in zero_outs]

    def run_prepacked(by_name):
        concat_in = [by_name[nm] for nm in in_names]
        zs = _STATE.pop("zeros_dev", None)
        if zs is None:
            zs = _put_zeros()
        out_arrs = sharded(*concat_in, *zs)
        res = [
            {nm: np.asarray(out_arrs[i]).reshape(
                NCORES, *out_avals[i].shape)[c]
             for i, nm in enumerate(out_names)}
            for c in range(NCORES)
        ]
        # replenish asynchronously; only the enqueue is paid here
        _STATE["zeros_dev"] = [jax.device_put(z, shardspec)
                               for z in zero_outs]
        return res

    class _Runner:
        prepacked = staticmethod(run_prepacked)
        put = staticmethod(lambda arr: jax.device_put(arr, shardspec))

    _STATE["runner"] = _Runner
    return _Runner


def _warm():
    """Trigger neuron compile + PJRT executable load with dummy inputs."""
    try:
        nc = _get_nc()
        a = {}
        a["times"] = np.arange(T, dtype=np.float32)
        for nm, sh in [("coeff_a", (B, N, T - 1, IN)),
                       ("coeff_b", (B, N, T - 1, IN)),
                       ("coeff_c2", (B, N, T - 1, IN)),
                       ("coeff_d3", (B, N, T - 1, IN)),
                       ("Wh", (IN, HID)), ("bh", (HID,)),
                       ("Wz", (IN, HID)), ("bz", (HID,)),
                       ("fWin", (HID, HH)), ("fbin", (HH,)),
                       ("fWmid", (HH, HH)), ("fbmid", (HH,)),
                       ("fWout", (HH, HID * IN)), ("fbout", (HID * IN,)),
                       ("gWin", (HID, HH)), ("gbin", (HH,)),
                       ("gE", (N, EMB)), ("gWpool", (EMB, K, HH, HH)),
                       ("gbpool", (EMB, HH)), ("gWout", (HH, HID * HID)),
                       ("gbout", (HID * HID,)), ("convW", (OUT, HID)),
                       ("convb", (OUT,))]:
            a[nm] = np.zeros(sh, np.float32)
        run = _get_runner()
        for _ in range(2):
            run.prepacked(host_inputs(a, overlap_put=run.put))
        _STATE["warm"] = True
    except Exception as e:  # pragma: no cover - keep import usable
        import traceback
        traceback.print_exc()
        _STATE["warm_err"] = e


def kernel(**inputs):
    run = _get_runner()
    a = {k_: np.asarray(v, dtype=np.float32) for k_, v in inputs.items()}
    results = run.prepacked(host_inputs(a, overlap_put=run.put))
    full = np.empty((B, 1, N, OUT), np.float32)
    for c in range(NCORES):
        o = np.asarray(results[c]["out"]).astype(np.float32)  # (OUT, R)
        full[c * BS:(c + 1) * BS, 0] = (
            o.reshape(OUT, BS, N).transpose(1, 2, 0))
    return full


_warm()
